# revision 21
# baseline (speedup 1.0000x reference)
"""Block-diagonal matmul with softmax-normalized weights, SPMD on 8 NeuronCores.

Computes: out[b, n*128+o] = sum_m x[b, n*128+m] * softmax(c[n], axis=m)[m, o]
for n in 512 independent 128x128 blocks, b in 2048 batch rows.

Sharding: blocks are fully independent -> shard the n_blocks axis across the
8 cores (64 blocks per core). Each core sees x columns [i*8192, (i+1)*8192),
blocks c[i*64:(i+1)*64], and produces the matching output column slice.

Layout + dtype strategy (rel-err budget is 2e-2; this path lands at ~1.4e-2,
validated at full scale on the exact arithmetic chain):
  - x is host-repacked per core to m-major bf16 [m=128, n*b] so the
    contraction dim m sits on SBUF partitions directly -- no PE transposes at
    all (they cost as much PE time as the matmuls themselves in the fp32
    version) -- and DMA traffic is halved vs fp32.
  - c is host-repacked to m-major bf16 [m=128, n*o] (one 2 MiB DMA with
    16 KiB per-partition rows).
  - Output is produced transposed ([o=128, n*b] bf16) straight from the
    matmul's natural PSUM orientation, DMA'd out in bf16 (halving write
    traffic), and untransposed/upcast on the host.

Per-core kernel (Tile framework). The run is DMA-bound (~66 MiB at the
~320-340 GB/s effective mixed-read/write HBM rate per core -> ~200 us), so the
whole design keeps the eviction engines (VectorE + ScalarE) and the DMA queues
free of anything serializing:
  - Softmax normalization never touches the weights: the matmul uses the
    UNNORMALIZED e = exp(c) (bf16, straight out of ScalarE, one table load) as
    the stationary operand, and the 1/colsum correction is folded into the
    PSUM eviction as a free per-partition scalar multiply (the block-n output
    sits in PSUM as [o, b], and 1/colsum[n, o] is constant along b). Earlier
    versions that normalized w up front serialized a 23 us reciprocal/multiply
    chain at the head of the VectorE queue, head-of-line blocking all PSUM
    evictions behind it and starving the DMA stream for ~25 us.
  - colsum(e_n) comes from a tiny N=1 matmul e_n^T @ ones per block, batched
    per 4-block group into one PSUM bank; one ~100 ns DVE reciprocal
    (Newton-Raphson approx, ~18 correct bits, colsums are ~128*E[exp] so no
    edge cases) turns each group's [o, 4] sums into scales. These are emitted
    lazily at group boundaries inside the phase-2 loop so they never dam up
    the PE/DVE queues.
  - Phase 2 is weight-stationary: for each block n, e_n [m,o] is the 128x128
    stationary operand and the whole 2048-col batch of xT_n [m,b] streams
    through as 4 N=512 bf16 matmuls into PSUM [o,b]. Evictions (PSUM fp32 ->
    SBUF bf16 with the scale) run at ~1.3 us per 2-bank half-block and total
    ~170 us -- more than one engine's worth -- so they alternate between
    VectorE and ScalarE (the first few stay on VectorE while ScalarE finishes
    the 16 Exps; ScalarE ops are issued Exp-batched because every activation
    function switch costs a 1.3 us ACT_TABLE_LOAD).
  - 4 MiB DMAs stream x in and 2 MiB DMAs stream results out (long
    per-partition bursts, few ~1 us dispatches, all on the sync queue; the c
    DMA dispatches on the scalar queue so it does not delay the first x).
"""

import numpy as np
from contextlib import ExitStack

import ml_dtypes

import concourse.bacc as bacc
import concourse.tile as tile
from concourse import mybir
from concourse.bass_utils import run_bass_kernel_spmd

F32 = mybir.dt.float32
BF16 = mybir.dt.bfloat16
BF16_NP = ml_dtypes.bfloat16
P = 128
N_CORES = 8
N_BLOCKS_TOTAL = 512
BLOCKS_PER_CORE = N_BLOCKS_TOTAL // N_CORES  # 64
BATCH = 2048
XCOLS = BLOCKS_PER_CORE * P  # 8192
LAYER = N_BLOCKS_TOTAL * P   # 65536


def _body(tc, out, x, c, batch, blocks):
    nc = tc.nc
    G1 = 4                         # blocks per sum group (one PSUM bank)
    OCHUNK = min(2, blocks)        # blocks per out DMA (1 MiB bf16)
    NMM = min(512, batch)          # moving cols per matmul (one fp32 bank)
    PS_COLS = min(1024, batch)     # psum tile cols (two banks)
    n_groups = blocks // G1
    n_evict = blocks * (batch // PS_COLS)
    # Early evictions stay on DVE while ScalarE finishes the Exps; after that,
    # alternate so the eviction work splits across both engines.
    ev_dve_only = 6
    # x DMA ramp: small chunks at the head so the matmul/eviction/out
    # pipeline starts ~15 us earlier than a uniform 8-block chunking would
    # allow, then 4 MiB chunks for long HBM read bursts. (A matching tail
    # ramp was tried and REGRESSED: the final small chunks arrive serialized
    # on xp2 buffer recycling, while a regular 8-block final chunk is
    # prefetched ~20 us before its matmuls need it.)
    if blocks >= 16:
        x_chunks = [2] * 4 + [8] * ((blocks - 8) // 8)
    elif blocks >= 8:
        x_chunks = [2] * (blocks // 2)
    else:
        x_chunks = [blocks]
    assert sum(x_chunks) == blocks

    def evict(i, out_ap, psum_ap, scale_ap):
        """PSUM fp32 -> SBUF bf16 with the softmax normalization folded in as
        a per-partition scalar multiply."""
        if i < ev_dve_only or i % 2 == 0:
            nc.vector.tensor_scalar_mul(out_ap, psum_ap, scale_ap)
        else:
            nc.scalar.mul(out_ap, psum_ap, scale_ap)

    with ExitStack() as ctx:
        const = ctx.enter_context(tc.tile_pool(name="const", bufs=1))
        ones_sb = const.tile([P, P], BF16)
        nc.vector.memset(ones_sb[:], 1.0)
        # Unnormalized weights e = exp(c), one tile per 4-block group.
        wpool = ctx.enter_context(tc.tile_pool(name="wpool", bufs=1))
        w_tiles = [wpool.tile([P, G1 * P], BF16, name=f"w{g}", tag=f"w{g}")
                   for g in range(n_groups)]
        # Per-group eviction scales rv[o, r] = 1/colsum(block g*4+r, o).
        rvpool = ctx.enter_context(tc.tile_pool(name="rvpool", bufs=1))
        rv_tiles = [rvpool.tile([P, G1], F32, name=f"rv{g}", tag=f"rv{g}")
                    for g in range(n_groups)]

        def w_slice(n):
            """AP for block n's unnormalized weights [m, o]."""
            g, r = divmod(n, G1)
            return w_tiles[g][:, r * P:(r + 1) * P]

        xpools = {}
        for sz, bufs in ((2, 2), (8, 4)):
            if sz in x_chunks:
                xpools[sz] = ctx.enter_context(
                    tc.tile_pool(name=f"xpool{sz}", bufs=bufs))
        opool = ctx.enter_context(tc.tile_pool(name="opool", bufs=4))
        psum_o = ctx.enter_context(tc.tile_pool(name="psum_o", bufs=3, space="PSUM"))
        psum_s = ctx.enter_context(tc.tile_pool(name="psum_s", bufs=2, space="PSUM"))
        cpool = ctx.enter_context(tc.tile_pool(name="cpool", bufs=2))

        # ---- Phase 1: e = exp(c), Exp-batched on ScalarE ----
        # The c DMA is split into pieces so the first Exps run as soon as the
        # first piece lands (a single 2 MiB c DMA interleaves with the x
        # stream at descriptor granularity and only completes at ~24 us,
        # which delayed the entire pipeline). Scalar-queue HWDGE: dispatches
        # run concurrently with the x dispatches on the sync queue.
        CPIECE = max(1, n_groups // 4)  # groups per c piece
        c_tiles = []
        # The pieces share one tag (2 rotating buffers): piece k reuses piece
        # k-2's buffer once its Exps have read it, saving 8 KiB of SBUF.
        for p0 in range(0, n_groups, CPIECE):
            ct = cpool.tile([P, CPIECE * G1 * P], BF16, name=f"c{p0}",
                            tag="cpiece")
            nc.scalar.dma_start(
                out=ct[:],
                in_=c[:, p0 * G1 * P:(p0 + CPIECE) * G1 * P],
            )
            c_tiles.append(ct)
            for g in range(p0, p0 + CPIECE):
                nc.scalar.activation(
                    w_tiles[g][:],
                    ct[:, (g - p0) * G1 * P:(g - p0 + 1) * G1 * P],
                    mybir.ActivationFunctionType.Exp)

        # ---- Phase 2: weight-stationary block matmuls over the full batch,
        # with the per-group colsum/reciprocal emitted lazily at group
        # boundaries so no engine queue is dammed up at the start.
        ev = 0
        groups_done = set()
        nb0 = 0
        for XCHUNK in x_chunks:
            gx0 = nb0
            xt = xpools[XCHUNK].tile([P, XCHUNK * batch], BF16)
            nc.sync.dma_start(
                out=xt[:],
                in_=x[:, gx0 * batch:(gx0 + XCHUNK) * batch],
            )
            nb0 += XCHUNK
            for jo in range(XCHUNK // OCHUNK):
                ot = opool.tile([P, OCHUNK * batch], BF16)
                for j2 in range(OCHUNK):
                    j = jo * OCHUNK + j2
                    n = gx0 + j
                    g, r = divmod(n, G1)
                    if g not in groups_done:
                        groups_done.add(g)
                        ps = psum_s.tile([P, G1], F32)
                        for rr in range(G1):
                            nc.tensor.matmul(
                                ps[:, rr:rr + 1],
                                w_slice(g * G1 + rr),
                                ones_sb[:, 0:1],
                                start=True,
                                stop=True,
                            )
                        nc.vector.reciprocal_approx_fast(
                            out=rv_tiles[g][:], in_=ps[:])
                    for h in range(batch // PS_COLS):
                        pso = psum_o.tile([P, PS_COLS], F32)
                        for k in range(PS_COLS // NMM):
                            col = h * PS_COLS + k * NMM
                            nc.tensor.matmul(
                                pso[:, k * NMM:(k + 1) * NMM],
                                w_slice(n),
                                xt[:, j * batch + col:j * batch + col + NMM],
                                start=True,
                                stop=True,
                            )
                        evict(
                            ev,
                            ot[:, j2 * batch + h * PS_COLS:
                               j2 * batch + (h + 1) * PS_COLS],
                            pso[:],
                            rv_tiles[g][:, r:r + 1],
                        )
                        ev += 1
                nc.sync.dma_start(
                    out=out[:, (gx0 + jo * OCHUNK) * batch:
                            (gx0 + (jo + 1) * OCHUNK) * batch],
                    in_=ot[:],
                )


def build_program(batch=BATCH, blocks=BLOCKS_PER_CORE):
    nc = bacc.Bacc("TRN2", target_bir_lowering=False, debug=False)
    # x arrives host-repacked as m-major bf16 [m, n*b], see repack_x.
    x = nc.dram_tensor("x", [P, blocks * batch], BF16, kind="ExternalInput").ap()
    # c arrives host-repacked as m-major bf16 [m, n*o], see repack_c.
    c = nc.dram_tensor("c", [P, blocks * P], BF16, kind="ExternalInput").ap()
    # out leaves o-major bf16 [o, n*b], untransposed on host, see unpack_out.
    out = nc.dram_tensor("out", [P, blocks * batch], BF16, kind="ExternalOutput").ap()
    with tile.TileContext(nc) as tc:
        _body(tc, out, x, c, batch, blocks)
    nc.compile()
    return nc


_NC_CACHE = {}


def _get_nc():
    if "nc" not in _NC_CACHE:
        _NC_CACHE["nc"] = build_program()
    return _NC_CACHE["nc"]


def repack_c(c_shard):
    """[n, m, o] -> m-major bf16 [m, n*o]: one efficient DMA, half the bytes.

    bf16 c moves the end-to-end rel err from ~6e-3 to ~1.4e-2 (validated at
    full scale on the exact device arithmetic path) -- still 1.4x under the
    2e-2 budget -- and saves ~6.5 us of DMA."""
    n = c_shard.shape[0]
    return (
        c_shard.transpose(1, 0, 2)
        .astype(BF16_NP)
        .reshape(P, n * P)
    )


def repack_x(x_shard):
    """[b, n*m] fp32 -> m-major bf16 [m, n*b]: contraction dim on partitions."""
    batch, cols = x_shard.shape
    nb = cols // P
    return (
        x_shard.reshape(batch, nb, P)
        .transpose(2, 1, 0)
        .astype(BF16_NP)
        .reshape(P, nb * batch)
    )


def unpack_out(o_packed, batch, blocks):
    """o-major bf16 [o, n*b] -> [b, n*o] fp32."""
    return (
        np.asarray(o_packed)
        .reshape(P, blocks, batch)
        .transpose(2, 1, 0)
        .astype(np.float32)
        .reshape(batch, blocks * P)
    )


def _make_in_maps(x, c):
    xr = x.reshape(BATCH, N_CORES, XCOLS)
    in_maps = []
    for i in range(N_CORES):
        in_maps.append(
            {
                "x": repack_x(np.ascontiguousarray(xr[:, i, :])),
                "c": repack_c(c[i * BLOCKS_PER_CORE:(i + 1) * BLOCKS_PER_CORE]),
            }
        )
    return in_maps


def run_on_hw(x, c, trace=False):
    """Run the SPMD kernel on the 8 cores; returns (out, BassKernelResults)."""
    x = np.asarray(x, dtype=np.float32)
    c = np.asarray(c, dtype=np.float32)
    assert x.shape == (BATCH, LAYER), x.shape
    assert c.shape == (N_BLOCKS_TOTAL, P, P), c.shape
    nc = _get_nc()
    in_maps = _make_in_maps(x, c)
    res = None
    for attempt in range(3):
        try:
            res = run_bass_kernel_spmd(
                nc, in_maps, core_ids=list(range(N_CORES)), trace=trace
            )
            break
        except Exception:
            # Transient runtime failures (e.g. a device flake) are rare but
            # fatal to a single attempt; retry with a fresh dispatch.
            if attempt == 2:
                raise
    assert res is not None
    out = np.empty((BATCH, LAYER), dtype=np.float32)
    orv = out.reshape(BATCH, N_CORES, XCOLS)
    for i in range(N_CORES):
        orv[:, i, :] = unpack_out(res.results[i]["out"], BATCH, BLOCKS_PER_CORE)
    return out, res


def kernel(x, c):
    out, _ = run_on_hw(x, c, trace=False)
    return out


# revision 22
# speedup vs baseline: 1.0160x; 1.0160x over previous
"""Block-diagonal matmul with softmax-normalized weights, SPMD on 8 NeuronCores.

Computes: out[b, n*128+o] = sum_m x[b, n*128+m] * softmax(c[n], axis=m)[m, o]
for n in 512 independent 128x128 blocks, b in 2048 batch rows.

Sharding: blocks are fully independent -> shard the n_blocks axis across the
8 cores (64 blocks per core). Each core sees x columns [i*8192, (i+1)*8192),
blocks c[i*64:(i+1)*64], and produces the matching output column slice.

Layout + dtype strategy (rel-err budget is 2e-2; this path lands at ~1.4e-2,
validated at full scale on the exact arithmetic chain):
  - x is host-repacked per core to m-major bf16 [m=128, n*b] so the
    contraction dim m sits on SBUF partitions directly -- no PE transposes at
    all (they cost as much PE time as the matmuls themselves in the fp32
    version) -- and DMA traffic is halved vs fp32.
  - c is host-repacked to m-major bf16 [m=128, n*o] (one 2 MiB DMA with
    16 KiB per-partition rows).
  - Output is produced transposed ([o=128, n*b] bf16) straight from the
    matmul's natural PSUM orientation, DMA'd out in bf16 (halving write
    traffic), and untransposed/upcast on the host.

Per-core kernel (Tile framework). The run is DMA-bound (~66 MiB at the
~320-340 GB/s effective mixed-read/write HBM rate per core -> ~200 us), so the
whole design keeps the eviction engines (VectorE + ScalarE) and the DMA queues
free of anything serializing:
  - Softmax normalization never touches the weights: the matmul uses the
    UNNORMALIZED e = exp(c) (bf16, straight out of ScalarE, one table load) as
    the stationary operand, and the 1/colsum correction is folded into the
    PSUM eviction as a free per-partition scalar multiply (the block-n output
    sits in PSUM as [o, b], and 1/colsum[n, o] is constant along b). Earlier
    versions that normalized w up front serialized a 23 us reciprocal/multiply
    chain at the head of the VectorE queue, head-of-line blocking all PSUM
    evictions behind it and starving the DMA stream for ~25 us.
  - colsum(e_n) comes from a tiny N=1 matmul e_n^T @ ones per block, batched
    per 4-block group into one PSUM bank; one ~100 ns DVE reciprocal
    (Newton-Raphson approx, ~18 correct bits, colsums are ~128*E[exp] so no
    edge cases) turns each group's [o, 4] sums into scales. These are emitted
    lazily at group boundaries inside the phase-2 loop so they never dam up
    the PE/DVE queues.
  - Phase 2 is weight-stationary: for each block n, e_n [m,o] is the 128x128
    stationary operand and the whole 2048-col batch of xT_n [m,b] streams
    through as 4 N=512 bf16 matmuls into PSUM [o,b]. Evictions (PSUM fp32 ->
    SBUF bf16 with the scale) run at ~1.3 us per 2-bank half-block and total
    ~170 us -- more than one engine's worth -- so they alternate between
    VectorE and ScalarE (the first few stay on VectorE while ScalarE finishes
    the 16 Exps; ScalarE ops are issued Exp-batched because every activation
    function switch costs a 1.3 us ACT_TABLE_LOAD).
  - 4 MiB DMAs stream x in and 2 MiB DMAs stream results out (long
    per-partition bursts, few ~1 us dispatches, all on the sync queue; the c
    DMA dispatches on the scalar queue so it does not delay the first x).
"""

import numpy as np
from contextlib import ExitStack

import ml_dtypes

import concourse.bacc as bacc
import concourse.tile as tile
from concourse import mybir
from concourse.bass_utils import run_bass_kernel_spmd

F32 = mybir.dt.float32
BF16 = mybir.dt.bfloat16
BF16_NP = ml_dtypes.bfloat16
P = 128
N_CORES = 8
N_BLOCKS_TOTAL = 512
BLOCKS_PER_CORE = N_BLOCKS_TOTAL // N_CORES  # 64
BATCH = 2048
XCOLS = BLOCKS_PER_CORE * P  # 8192
LAYER = N_BLOCKS_TOTAL * P   # 65536


def _body(tc, out, x, c, batch, blocks):
    nc = tc.nc
    G1 = 4                         # blocks per sum group (one PSUM bank)
    OCHUNK = min(2, blocks)        # blocks per out DMA (1 MiB bf16)
    NMM = min(512, batch)          # moving cols per matmul (one fp32 bank)
    PS_COLS = min(1024, batch)     # psum tile cols (two banks)
    n_groups = blocks // G1
    n_evict = blocks * (batch // PS_COLS)
    # Early evictions stay on DVE while ScalarE finishes the Exps; after that,
    # alternate so the eviction work splits across both engines.
    ev_dve_only = 6
    # x DMA ramp: small chunks at BOTH ends -- at the head so the
    # matmul/eviction/out pipeline starts ~15 us earlier than a uniform
    # 8-block chunking would allow, and at the tail so the final blocks'
    # x, matmuls, and evictions interleave with the out drain instead of
    # arriving as one 4 MiB wall (A/B-measured ~2x10 us better than a
    # uniform-8 tail across repeated runs). 4 MiB chunks in the middle for
    # long HBM read bursts.
    if blocks >= 16:
        x_chunks = [2] * 4 + [8] * ((blocks - 16) // 8) + [2] * 4
    elif blocks >= 8:
        x_chunks = [2] * (blocks // 2)
    else:
        x_chunks = [blocks]
    assert sum(x_chunks) == blocks

    def evict(i, out_ap, psum_ap, scale_ap):
        """PSUM fp32 -> SBUF bf16 with the softmax normalization folded in as
        a per-partition scalar multiply."""
        if i < ev_dve_only or i % 2 == 0:
            nc.vector.tensor_scalar_mul(out_ap, psum_ap, scale_ap)
        else:
            nc.scalar.mul(out_ap, psum_ap, scale_ap)

    with ExitStack() as ctx:
        const = ctx.enter_context(tc.tile_pool(name="const", bufs=1))
        ones_sb = const.tile([P, P], BF16)
        nc.vector.memset(ones_sb[:], 1.0)
        # Unnormalized weights e = exp(c), one tile per 4-block group.
        wpool = ctx.enter_context(tc.tile_pool(name="wpool", bufs=1))
        w_tiles = [wpool.tile([P, G1 * P], BF16, name=f"w{g}", tag=f"w{g}")
                   for g in range(n_groups)]
        # Per-group eviction scales rv[o, r] = 1/colsum(block g*4+r, o).
        rvpool = ctx.enter_context(tc.tile_pool(name="rvpool", bufs=1))
        rv_tiles = [rvpool.tile([P, G1], F32, name=f"rv{g}", tag=f"rv{g}")
                    for g in range(n_groups)]

        def w_slice(n):
            """AP for block n's unnormalized weights [m, o]."""
            g, r = divmod(n, G1)
            return w_tiles[g][:, r * P:(r + 1) * P]

        xpools = {}
        for sz, bufs in ((2, 2), (8, 4)):
            if sz in x_chunks:
                xpools[sz] = ctx.enter_context(
                    tc.tile_pool(name=f"xpool{sz}", bufs=bufs))
        opool = ctx.enter_context(tc.tile_pool(name="opool", bufs=4))
        psum_o = ctx.enter_context(tc.tile_pool(name="psum_o", bufs=3, space="PSUM"))
        psum_s = ctx.enter_context(tc.tile_pool(name="psum_s", bufs=2, space="PSUM"))
        cpool = ctx.enter_context(tc.tile_pool(name="cpool", bufs=2))

        # ---- Phase 1: e = exp(c), Exp-batched on ScalarE ----
        # The c DMA is split into pieces so the first Exps run as soon as the
        # first piece lands (a single 2 MiB c DMA interleaves with the x
        # stream at descriptor granularity and only completes at ~24 us,
        # which delayed the entire pipeline). Scalar-queue HWDGE: dispatches
        # run concurrently with the x dispatches on the sync queue.
        CPIECE = max(1, n_groups // 4)  # groups per c piece
        c_tiles = []
        # The pieces share one tag (2 rotating buffers): piece k reuses piece
        # k-2's buffer once its Exps have read it, saving 8 KiB of SBUF.
        for p0 in range(0, n_groups, CPIECE):
            ct = cpool.tile([P, CPIECE * G1 * P], BF16, name=f"c{p0}",
                            tag="cpiece")
            nc.scalar.dma_start(
                out=ct[:],
                in_=c[:, p0 * G1 * P:(p0 + CPIECE) * G1 * P],
            )
            c_tiles.append(ct)
            for g in range(p0, p0 + CPIECE):
                nc.scalar.activation(
                    w_tiles[g][:],
                    ct[:, (g - p0) * G1 * P:(g - p0 + 1) * G1 * P],
                    mybir.ActivationFunctionType.Exp)

        # ---- Phase 2: weight-stationary block matmuls over the full batch,
        # with the per-group colsum/reciprocal emitted lazily at group
        # boundaries so no engine queue is dammed up at the start.
        ev = 0
        groups_done = set()
        nb0 = 0
        for XCHUNK in x_chunks:
            gx0 = nb0
            xt = xpools[XCHUNK].tile([P, XCHUNK * batch], BF16)
            nc.sync.dma_start(
                out=xt[:],
                in_=x[:, gx0 * batch:(gx0 + XCHUNK) * batch],
            )
            nb0 += XCHUNK
            for jo in range(XCHUNK // OCHUNK):
                ot = opool.tile([P, OCHUNK * batch], BF16)
                for j2 in range(OCHUNK):
                    j = jo * OCHUNK + j2
                    n = gx0 + j
                    g, r = divmod(n, G1)
                    if g not in groups_done:
                        groups_done.add(g)
                        ps = psum_s.tile([P, G1], F32)
                        for rr in range(G1):
                            nc.tensor.matmul(
                                ps[:, rr:rr + 1],
                                w_slice(g * G1 + rr),
                                ones_sb[:, 0:1],
                                start=True,
                                stop=True,
                            )
                        nc.vector.reciprocal_approx_fast(
                            out=rv_tiles[g][:], in_=ps[:])
                    for h in range(batch // PS_COLS):
                        pso = psum_o.tile([P, PS_COLS], F32)
                        for k in range(PS_COLS // NMM):
                            col = h * PS_COLS + k * NMM
                            nc.tensor.matmul(
                                pso[:, k * NMM:(k + 1) * NMM],
                                w_slice(n),
                                xt[:, j * batch + col:j * batch + col + NMM],
                                start=True,
                                stop=True,
                            )
                        evict(
                            ev,
                            ot[:, j2 * batch + h * PS_COLS:
                               j2 * batch + (h + 1) * PS_COLS],
                            pso[:],
                            rv_tiles[g][:, r:r + 1],
                        )
                        ev += 1
                nc.sync.dma_start(
                    out=out[:, (gx0 + jo * OCHUNK) * batch:
                            (gx0 + (jo + 1) * OCHUNK) * batch],
                    in_=ot[:],
                )


def build_program(batch=BATCH, blocks=BLOCKS_PER_CORE):
    nc = bacc.Bacc("TRN2", target_bir_lowering=False, debug=False)
    # x arrives host-repacked as m-major bf16 [m, n*b], see repack_x.
    x = nc.dram_tensor("x", [P, blocks * batch], BF16, kind="ExternalInput").ap()
    # c arrives host-repacked as m-major bf16 [m, n*o], see repack_c.
    c = nc.dram_tensor("c", [P, blocks * P], BF16, kind="ExternalInput").ap()
    # out leaves o-major bf16 [o, n*b], untransposed on host, see unpack_out.
    out = nc.dram_tensor("out", [P, blocks * batch], BF16, kind="ExternalOutput").ap()
    with tile.TileContext(nc) as tc:
        _body(tc, out, x, c, batch, blocks)
    nc.compile()
    return nc


_NC_CACHE = {}


def _get_nc():
    if "nc" not in _NC_CACHE:
        _NC_CACHE["nc"] = build_program()
    return _NC_CACHE["nc"]


def repack_c(c_shard):
    """[n, m, o] -> m-major bf16 [m, n*o]: one efficient DMA, half the bytes.

    bf16 c moves the end-to-end rel err from ~6e-3 to ~1.4e-2 (validated at
    full scale on the exact device arithmetic path) -- still 1.4x under the
    2e-2 budget -- and saves ~6.5 us of DMA."""
    n = c_shard.shape[0]
    return (
        c_shard.transpose(1, 0, 2)
        .astype(BF16_NP)
        .reshape(P, n * P)
    )


def repack_x(x_shard):
    """[b, n*m] fp32 -> m-major bf16 [m, n*b]: contraction dim on partitions."""
    batch, cols = x_shard.shape
    nb = cols // P
    return (
        x_shard.reshape(batch, nb, P)
        .transpose(2, 1, 0)
        .astype(BF16_NP)
        .reshape(P, nb * batch)
    )


def unpack_out(o_packed, batch, blocks):
    """o-major bf16 [o, n*b] -> [b, n*o] fp32."""
    return (
        np.asarray(o_packed)
        .reshape(P, blocks, batch)
        .transpose(2, 1, 0)
        .astype(np.float32)
        .reshape(batch, blocks * P)
    )


def _make_in_maps(x, c):
    xr = x.reshape(BATCH, N_CORES, XCOLS)
    in_maps = []
    for i in range(N_CORES):
        in_maps.append(
            {
                "x": repack_x(np.ascontiguousarray(xr[:, i, :])),
                "c": repack_c(c[i * BLOCKS_PER_CORE:(i + 1) * BLOCKS_PER_CORE]),
            }
        )
    return in_maps


def run_on_hw(x, c, trace=False):
    """Run the SPMD kernel on the 8 cores; returns (out, BassKernelResults)."""
    x = np.asarray(x, dtype=np.float32)
    c = np.asarray(c, dtype=np.float32)
    assert x.shape == (BATCH, LAYER), x.shape
    assert c.shape == (N_BLOCKS_TOTAL, P, P), c.shape
    nc = _get_nc()
    in_maps = _make_in_maps(x, c)
    res = None
    for attempt in range(3):
        try:
            res = run_bass_kernel_spmd(
                nc, in_maps, core_ids=list(range(N_CORES)), trace=trace
            )
            break
        except Exception:
            # Transient runtime failures (e.g. a device flake) are rare but
            # fatal to a single attempt; retry with a fresh dispatch.
            if attempt == 2:
                raise
    assert res is not None
    out = np.empty((BATCH, LAYER), dtype=np.float32)
    orv = out.reshape(BATCH, N_CORES, XCOLS)
    for i in range(N_CORES):
        orv[:, i, :] = unpack_out(res.results[i]["out"], BATCH, BLOCKS_PER_CORE)
    return out, res


def kernel(x, c):
    out, _ = run_on_hw(x, c, trace=False)
    return out


# revision 26
# speedup vs baseline: 1.0308x; 1.0147x over previous
"""Block-diagonal matmul with softmax-normalized weights, SPMD on 8 NeuronCores.

Computes: out[b, n*128+o] = sum_m x[b, n*128+m] * softmax(c[n], axis=m)[m, o]
for n in 512 independent 128x128 blocks, b in 2048 batch rows.

Sharding: blocks are fully independent -> shard the n_blocks axis across the
8 cores (64 blocks per core). Each core sees x columns [i*8192, (i+1)*8192),
blocks c[i*64:(i+1)*64], and produces the matching output column slice.

Layout + dtype strategy (rel-err budget is 2e-2; this path lands at ~1.4e-2,
validated at full scale on the exact arithmetic chain):
  - x is host-repacked per core to m-major bf16 [m=128, n*b] so the
    contraction dim m sits on SBUF partitions directly -- no PE transposes at
    all (they cost as much PE time as the matmuls themselves in the fp32
    version) -- and DMA traffic is halved vs fp32.
  - c is host-repacked to m-major bf16 [m=128, n*o] (one 2 MiB DMA with
    16 KiB per-partition rows).
  - Output is produced transposed ([o=128, n*b] bf16) straight from the
    matmul's natural PSUM orientation, DMA'd out in bf16 (halving write
    traffic), and untransposed/upcast on the host.

Per-core kernel (Tile framework). The run is DMA-bound (~66 MiB at the
~320-340 GB/s effective mixed-read/write HBM rate per core -> ~200 us), so the
whole design keeps the eviction engines (VectorE + ScalarE) and the DMA queues
free of anything serializing:
  - Softmax normalization never touches the weights: the matmul uses the
    UNNORMALIZED e = exp(c) (bf16, straight out of ScalarE, one table load) as
    the stationary operand, and the 1/colsum correction is folded into the
    PSUM eviction as a free per-partition scalar multiply (the block-n output
    sits in PSUM as [o, b], and 1/colsum[n, o] is constant along b). Earlier
    versions that normalized w up front serialized a 23 us reciprocal/multiply
    chain at the head of the VectorE queue, head-of-line blocking all PSUM
    evictions behind it and starving the DMA stream for ~25 us.
  - colsum(e_n) comes from a tiny N=1 matmul e_n^T @ ones per block, batched
    per 4-block group into one PSUM bank; one ~100 ns DVE reciprocal
    (Newton-Raphson approx, ~18 correct bits, colsums are ~128*E[exp] so no
    edge cases) turns each group's [o, 4] sums into scales. These are emitted
    lazily at group boundaries inside the phase-2 loop so they never dam up
    the PE/DVE queues.
  - Phase 2 is weight-stationary: for each block n, e_n [m,o] is the 128x128
    stationary operand and the whole 2048-col batch of xT_n [m,b] streams
    through as 4 N=512 bf16 matmuls into PSUM [o,b]. Evictions (PSUM fp32 ->
    SBUF bf16 with the scale) run at ~1.3 us per 2-bank half-block and total
    ~170 us -- more than one engine's worth -- so they alternate between
    VectorE and ScalarE (the first few stay on VectorE while ScalarE finishes
    the 16 Exps; ScalarE ops are issued Exp-batched because every activation
    function switch costs a 1.3 us ACT_TABLE_LOAD).
  - DMA queue discipline is the difference between ~204 us and ~186 us: a
    dma_start dispatch WAITS for its input-tile dependencies before
    generating descriptors, and engine queues are strict FIFO, so an out
    dispatch parked on the sync queue head-of-line blocks every x prefetch
    dispatch behind it. The sync queue therefore carries ONLY x (ramped
    2-block chunks at both ends, 4 MiB in the middle); out DMAs alternate
    between the otherwise-idle GpSimd SWDGE queue and the Scalar queue, and
    the c pieces dispatch on the Scalar queue ahead of the Exps that consume
    them. With that, the 16 DMA engines measure 100% busy from ~10 us to
    ~180 us of a ~188 us run -- the saturated-DMA floor for ~66 MiB at the
    ~390 GB/s effective mixed rate.
"""

import numpy as np
from contextlib import ExitStack

import ml_dtypes

import concourse.bacc as bacc
import concourse.tile as tile
from concourse import mybir
from concourse.bass_utils import run_bass_kernel_spmd

F32 = mybir.dt.float32
BF16 = mybir.dt.bfloat16
BF16_NP = ml_dtypes.bfloat16
P = 128
N_CORES = 8
N_BLOCKS_TOTAL = 512
BLOCKS_PER_CORE = N_BLOCKS_TOTAL // N_CORES  # 64
BATCH = 2048
XCOLS = BLOCKS_PER_CORE * P  # 8192
LAYER = N_BLOCKS_TOTAL * P   # 65536


def _body(tc, out, x, c, batch, blocks):
    nc = tc.nc
    G1 = 4                         # blocks per sum group (one PSUM bank)
    OCHUNK = min(2, blocks)        # blocks per out DMA (1 MiB bf16)
    NMM = min(512, batch)          # moving cols per matmul (one fp32 bank)
    PS_COLS = min(1024, batch)     # psum tile cols (two banks)
    n_groups = blocks // G1
    n_evict = blocks * (batch // PS_COLS)
    # Early evictions stay on DVE while ScalarE finishes the Exps; after that,
    # alternate so the eviction work splits across both engines.
    ev_dve_only = 6
    # x DMA ramp: small chunks at BOTH ends -- at the head so the
    # matmul/eviction/out pipeline starts ~15 us earlier than a uniform
    # 8-block chunking would allow, and at the tail so the final blocks'
    # x, matmuls, and evictions interleave with the out drain instead of
    # arriving as one 4 MiB wall (A/B-measured ~2x10 us better than a
    # uniform-8 tail across repeated runs). 4 MiB chunks in the middle for
    # long HBM read bursts.
    if blocks >= 16:
        x_chunks = [2] * 4 + [8] * ((blocks - 16) // 8) + [2] * 4
    elif blocks >= 8:
        x_chunks = [2] * (blocks // 2)
    else:
        x_chunks = [blocks]
    assert sum(x_chunks) == blocks

    def evict(i, out_ap, psum_ap, scale_ap):
        """PSUM fp32 -> SBUF bf16 with the softmax normalization folded in as
        a per-partition scalar multiply."""
        if i < ev_dve_only or i % 2 == 0:
            nc.vector.tensor_scalar_mul(out_ap, psum_ap, scale_ap)
        else:
            nc.scalar.mul(out_ap, psum_ap, scale_ap)

    with ExitStack() as ctx:
        const = ctx.enter_context(tc.tile_pool(name="const", bufs=1))
        ones_sb = const.tile([P, P], BF16)
        nc.vector.memset(ones_sb[:], 1.0)
        # Unnormalized weights e = exp(c), one tile per 4-block group.
        wpool = ctx.enter_context(tc.tile_pool(name="wpool", bufs=1))
        w_tiles = [wpool.tile([P, G1 * P], BF16, name=f"w{g}", tag=f"w{g}")
                   for g in range(n_groups)]
        # Per-group eviction scales rv[o, r] = 1/colsum(block g*4+r, o).
        rvpool = ctx.enter_context(tc.tile_pool(name="rvpool", bufs=1))
        rv_tiles = [rvpool.tile([P, G1], F32, name=f"rv{g}", tag=f"rv{g}")
                    for g in range(n_groups)]

        def w_slice(n):
            """AP for block n's unnormalized weights [m, o]."""
            g, r = divmod(n, G1)
            return w_tiles[g][:, r * P:(r + 1) * P]

        xpools = {}
        for sz, bufs in ((2, 2), (8, 4)):
            if sz in x_chunks:
                xpools[sz] = ctx.enter_context(
                    tc.tile_pool(name=f"xpool{sz}", bufs=bufs))
        opool = ctx.enter_context(tc.tile_pool(name="opool", bufs=4))
        psum_o = ctx.enter_context(tc.tile_pool(name="psum_o", bufs=3, space="PSUM"))
        psum_s = ctx.enter_context(tc.tile_pool(name="psum_s", bufs=2, space="PSUM"))
        cpool = ctx.enter_context(tc.tile_pool(name="cpool", bufs=2))

        # ---- Phase 1: e = exp(c), Exp-batched on ScalarE ----
        # The c DMA is split into pieces so the first Exps run as soon as the
        # first piece lands (a single 2 MiB c DMA interleaves with the x
        # stream at descriptor granularity and only completes at ~24 us,
        # which delayed the entire pipeline). Scalar-queue HWDGE: dispatches
        # run concurrently with the x dispatches on the sync queue.
        CPIECE = max(1, n_groups // 4)  # groups per c piece
        c_tiles = []
        # The pieces share one tag (2 rotating buffers): piece k reuses piece
        # k-2's buffer once its Exps have read it, saving 8 KiB of SBUF.
        for p0 in range(0, n_groups, CPIECE):
            ct = cpool.tile([P, CPIECE * G1 * P], BF16, name=f"c{p0}",
                            tag="cpiece")
            nc.scalar.dma_start(
                out=ct[:],
                in_=c[:, p0 * G1 * P:(p0 + CPIECE) * G1 * P],
            )
            c_tiles.append(ct)
            for g in range(p0, p0 + CPIECE):
                nc.scalar.activation(
                    w_tiles[g][:],
                    ct[:, (g - p0) * G1 * P:(g - p0 + 1) * G1 * P],
                    mybir.ActivationFunctionType.Exp)

        # ---- Phase 2: weight-stationary block matmuls over the full batch,
        # with the per-group colsum/reciprocal emitted lazily at group
        # boundaries so no engine queue is dammed up at the start.
        ev = 0
        groups_done = set()
        nb0 = 0
        for XCHUNK in x_chunks:
            gx0 = nb0
            xt = xpools[XCHUNK].tile([P, XCHUNK * batch], BF16)
            nc.sync.dma_start(
                out=xt[:],
                in_=x[:, gx0 * batch:(gx0 + XCHUNK) * batch],
            )
            nb0 += XCHUNK
            for jo in range(XCHUNK // OCHUNK):
                ot = opool.tile([P, OCHUNK * batch], BF16)
                for j2 in range(OCHUNK):
                    j = jo * OCHUNK + j2
                    n = gx0 + j
                    g, r = divmod(n, G1)
                    if g not in groups_done:
                        groups_done.add(g)
                        ps = psum_s.tile([P, G1], F32)
                        for rr in range(G1):
                            nc.tensor.matmul(
                                ps[:, rr:rr + 1],
                                w_slice(g * G1 + rr),
                                ones_sb[:, 0:1],
                                start=True,
                                stop=True,
                            )
                        nc.vector.reciprocal_approx_fast(
                            out=rv_tiles[g][:], in_=ps[:])
                    for h in range(batch // PS_COLS):
                        pso = psum_o.tile([P, PS_COLS], F32)
                        for k in range(PS_COLS // NMM):
                            col = h * PS_COLS + k * NMM
                            nc.tensor.matmul(
                                pso[:, k * NMM:(k + 1) * NMM],
                                w_slice(n),
                                xt[:, j * batch + col:j * batch + col + NMM],
                                start=True,
                                stop=True,
                            )
                        evict(
                            ev,
                            ot[:, j2 * batch + h * PS_COLS:
                               j2 * batch + (h + 1) * PS_COLS],
                            pso[:],
                            rv_tiles[g][:, r:r + 1],
                        )
                        ev += 1
                # Out DMAs never dispatch on the sync queue mid-run: a
                # dma_start dispatch WAITS for its input tile's evictions
                # before generating descriptors, so on the sync queue each
                # out dispatch head-of-line blocked every later x prefetch
                # dispatch behind it (sync carries only x until the final
                # chunk). They alternate between the GpSimd SWDGE queue
                # (idle, ~2 us/dispatch) and the Scalar queue (rides between
                # ACT evictions, which have ~60 us of slack) so neither
                # queue's serialization paces the drain; the final chunk's
                # tile takes the by-then-idle sync queue.
                if nb0 >= blocks:
                    eng = nc.sync
                elif ev % 2 == 0:
                    eng = nc.gpsimd
                else:
                    eng = nc.scalar
                eng.dma_start(
                    out=out[:, (gx0 + jo * OCHUNK) * batch:
                            (gx0 + (jo + 1) * OCHUNK) * batch],
                    in_=ot[:],
                )


def build_program(batch=BATCH, blocks=BLOCKS_PER_CORE):
    nc = bacc.Bacc("TRN2", target_bir_lowering=False, debug=False)
    # x arrives host-repacked as m-major bf16 [m, n*b], see repack_x.
    x = nc.dram_tensor("x", [P, blocks * batch], BF16, kind="ExternalInput").ap()
    # c arrives host-repacked as m-major bf16 [m, n*o], see repack_c.
    c = nc.dram_tensor("c", [P, blocks * P], BF16, kind="ExternalInput").ap()
    # out leaves o-major bf16 [o, n*b], untransposed on host, see unpack_out.
    out = nc.dram_tensor("out", [P, blocks * batch], BF16, kind="ExternalOutput").ap()
    with tile.TileContext(nc) as tc:
        _body(tc, out, x, c, batch, blocks)
    nc.compile()
    return nc


_NC_CACHE = {}


def _get_nc():
    if "nc" not in _NC_CACHE:
        _NC_CACHE["nc"] = build_program()
    return _NC_CACHE["nc"]


def repack_c(c_shard):
    """[n, m, o] -> m-major bf16 [m, n*o]: one efficient DMA, half the bytes.

    bf16 c moves the end-to-end rel err from ~6e-3 to ~1.4e-2 (validated at
    full scale on the exact device arithmetic path) -- still 1.4x under the
    2e-2 budget -- and saves ~6.5 us of DMA."""
    n = c_shard.shape[0]
    return (
        c_shard.transpose(1, 0, 2)
        .astype(BF16_NP)
        .reshape(P, n * P)
    )


def repack_x(x_shard):
    """[b, n*m] fp32 -> m-major bf16 [m, n*b]: contraction dim on partitions."""
    batch, cols = x_shard.shape
    nb = cols // P
    return (
        x_shard.reshape(batch, nb, P)
        .transpose(2, 1, 0)
        .astype(BF16_NP)
        .reshape(P, nb * batch)
    )


def unpack_out(o_packed, batch, blocks):
    """o-major bf16 [o, n*b] -> [b, n*o] fp32."""
    return (
        np.asarray(o_packed)
        .reshape(P, blocks, batch)
        .transpose(2, 1, 0)
        .astype(np.float32)
        .reshape(batch, blocks * P)
    )


def _make_in_maps(x, c):
    xr = x.reshape(BATCH, N_CORES, XCOLS)
    in_maps = []
    for i in range(N_CORES):
        in_maps.append(
            {
                "x": repack_x(np.ascontiguousarray(xr[:, i, :])),
                "c": repack_c(c[i * BLOCKS_PER_CORE:(i + 1) * BLOCKS_PER_CORE]),
            }
        )
    return in_maps


def run_on_hw(x, c, trace=False):
    """Run the SPMD kernel on the 8 cores; returns (out, BassKernelResults)."""
    x = np.asarray(x, dtype=np.float32)
    c = np.asarray(c, dtype=np.float32)
    assert x.shape == (BATCH, LAYER), x.shape
    assert c.shape == (N_BLOCKS_TOTAL, P, P), c.shape
    nc = _get_nc()
    in_maps = _make_in_maps(x, c)
    res = None
    for attempt in range(3):
        try:
            res = run_bass_kernel_spmd(
                nc, in_maps, core_ids=list(range(N_CORES)), trace=trace
            )
            break
        except Exception:
            # Transient runtime failures (e.g. a device flake) are rare but
            # fatal to a single attempt; retry with a fresh dispatch.
            if attempt == 2:
                raise
    assert res is not None
    out = np.empty((BATCH, LAYER), dtype=np.float32)
    orv = out.reshape(BATCH, N_CORES, XCOLS)
    for i in range(N_CORES):
        orv[:, i, :] = unpack_out(res.results[i]["out"], BATCH, BLOCKS_PER_CORE)
    return out, res


def kernel(x, c):
    out, _ = run_on_hw(x, c, trace=False)
    return out


# revision 30
# speedup vs baseline: 1.2599x; 1.2222x over previous
"""Block-diagonal matmul with softmax-normalized weights, SPMD on 8 NeuronCores.

Computes: out[b, n*128+o] = sum_m x[b, n*128+m] * softmax(c[n], axis=m)[m, o]
for n in 512 independent 128x128 blocks, b in 2048 batch rows.

Sharding: blocks are fully independent -> shard the n_blocks axis across the
8 cores (64 blocks per core). Each core sees x columns [i*8192, (i+1)*8192),
blocks c[i*64:(i+1)*64], and produces the matching output column slice.

Layout + dtype strategy (rel-err budget is 2e-2; this path lands at ~1.4e-2,
validated at full scale on the exact arithmetic chain):
  - x is host-repacked per core to m-major bf16 [m=128, n*b] so the
    contraction dim m sits on SBUF partitions directly -- no PE transposes at
    all (they cost as much PE time as the matmuls themselves in the fp32
    version) -- and DMA traffic is halved vs fp32.
  - c is host-repacked to m-major bf16 [m=128, n*o] (one 2 MiB DMA with
    16 KiB per-partition rows).
  - Output is produced transposed ([o=128, n*b] bf16) straight from the
    matmul's natural PSUM orientation, DMA'd out in bf16 (halving write
    traffic), and untransposed/upcast on the host.

Per-core kernel (Tile framework). The run is DMA-bound (~66 MiB at the
~320-340 GB/s effective mixed-read/write HBM rate per core -> ~200 us), so the
whole design keeps the eviction engines (VectorE + ScalarE) and the DMA queues
free of anything serializing:
  - Softmax normalization never touches the weights: the matmul uses the
    UNNORMALIZED e = exp(c) (bf16, straight out of ScalarE, one table load) as
    the stationary operand, and the 1/colsum correction is folded into the
    PSUM eviction as a free per-partition scalar multiply (the block-n output
    sits in PSUM as [o, b], and 1/colsum[n, o] is constant along b). Earlier
    versions that normalized w up front serialized a 23 us reciprocal/multiply
    chain at the head of the VectorE queue, head-of-line blocking all PSUM
    evictions behind it and starving the DMA stream for ~25 us.
  - colsum(e_n) comes from a tiny N=1 matmul e_n^T @ ones per block, batched
    per 4-block group into one PSUM bank; one ~100 ns DVE reciprocal
    (Newton-Raphson approx, ~18 correct bits, colsums are ~128*E[exp] so no
    edge cases) turns each group's [o, 4] sums into scales. These are emitted
    lazily at group boundaries inside the phase-2 loop so they never dam up
    the PE/DVE queues.
  - Phase 2 is weight-stationary: for each block n, e_n [m,o] is the 128x128
    stationary operand and the whole 2048-col batch of xT_n [m,b] streams
    through as 4 N=512 bf16 matmuls into PSUM [o,b]. Evictions (PSUM fp32 ->
    SBUF bf16 with the scale) run at ~1.3 us per 2-bank half-block and total
    ~170 us -- more than one engine's worth -- so they alternate between
    VectorE and ScalarE (the first few stay on VectorE while ScalarE finishes
    the 16 Exps; ScalarE ops are issued Exp-batched because every activation
    function switch costs a 1.3 us ACT_TABLE_LOAD).
  - DMA queue discipline is the difference between ~204 us and ~186 us: a
    dma_start dispatch WAITS for its input-tile dependencies before
    generating descriptors, and engine queues are strict FIFO, so an out
    dispatch parked on the sync queue head-of-line blocks every x prefetch
    dispatch behind it. The sync queue therefore carries ONLY x (ramped
    2-block chunks at both ends, 4 MiB in the middle); out DMAs alternate
    between the otherwise-idle GpSimd SWDGE queue and the Scalar queue, and
    the c pieces dispatch on the Scalar queue ahead of the Exps that consume
    them. With that, the 16 DMA engines measure 100% busy from ~10 us to
    ~180 us of a ~188 us run -- the saturated-DMA floor for ~66 MiB at the
    ~390 GB/s effective mixed rate.
"""

import numpy as np
from contextlib import ExitStack

import ml_dtypes

import concourse.bacc as bacc
import concourse.tile as tile
from concourse import mybir
from concourse.bass_utils import run_bass_kernel_spmd

F32 = mybir.dt.float32
BF16 = mybir.dt.bfloat16
BF16_NP = ml_dtypes.bfloat16
P = 128
N_CORES = 8
N_BLOCKS_TOTAL = 512
BLOCKS_PER_CORE = N_BLOCKS_TOTAL // N_CORES  # 64
BATCH = 2048
XCOLS = BLOCKS_PER_CORE * P  # 8192
LAYER = N_BLOCKS_TOTAL * P   # 65536


def _body(tc, out, x, c, batch, blocks):
    nc = tc.nc
    G1 = 4                         # blocks per sum group (one PSUM bank)
    OCHUNK = min(2, blocks)        # blocks per out DMA (1 MiB bf16)
    NMM = min(512, batch)          # moving cols per matmul (one fp32 bank)
    PS_COLS = min(1024, batch)     # psum tile cols (two banks)
    n_groups = blocks // G1
    n_evict = blocks * (batch // PS_COLS)
    # Early evictions stay on DVE while ScalarE finishes the Exps; after that,
    # alternate so the eviction work splits across both engines.
    ev_dve_only = 6
    # x DMA ramp: small chunks at BOTH ends -- at the head so the
    # matmul/eviction/out pipeline starts ~15 us earlier than a uniform
    # 8-block chunking would allow, and at the tail so the final blocks'
    # x, matmuls, and evictions interleave with the out drain instead of
    # arriving as one 4 MiB wall (A/B-measured ~2x10 us better than a
    # uniform-8 tail across repeated runs). 4 MiB chunks in the middle for
    # long HBM read bursts.
    if blocks >= 16:
        x_chunks = [2] * 4 + [8] * ((blocks - 16) // 8) + [2] * 4
    elif blocks >= 8:
        x_chunks = [2] * (blocks // 2)
    else:
        x_chunks = [blocks]
    assert sum(x_chunks) == blocks

    def evict(i, out_ap, psum_ap, scale_ap):
        """PSUM fp32 -> SBUF bf16 with the softmax normalization folded in as
        a per-partition scalar multiply."""
        if i < ev_dve_only or i % 2 == 0:
            nc.vector.tensor_scalar_mul(out_ap, psum_ap, scale_ap)
        else:
            nc.scalar.mul(out_ap, psum_ap, scale_ap)

    with ExitStack() as ctx:
        const = ctx.enter_context(tc.tile_pool(name="const", bufs=1))
        ones_sb = const.tile([P, P], BF16)
        nc.vector.memset(ones_sb[:], 1.0)
        # Unnormalized weights e = exp(c), one tile per 4-block group.
        wpool = ctx.enter_context(tc.tile_pool(name="wpool", bufs=1))
        w_tiles = [wpool.tile([P, G1 * P], BF16, name=f"w{g}", tag=f"w{g}")
                   for g in range(n_groups)]
        # Per-group eviction scales rv[o, r] = 1/colsum(block g*4+r, o).
        rvpool = ctx.enter_context(tc.tile_pool(name="rvpool", bufs=1))
        rv_tiles = [rvpool.tile([P, G1], F32, name=f"rv{g}", tag=f"rv{g}")
                    for g in range(n_groups)]

        def w_slice(n):
            """AP for block n's unnormalized weights [m, o]."""
            g, r = divmod(n, G1)
            return w_tiles[g][:, r * P:(r + 1) * P]

        xpools = {}
        # Buffer counts are A/B-measured optima: xp8 bufs=3 (to fund deeper
        # tail prefetch) lost ~5 us of mid-run saturation, and funding it
        # from cpool/opool instead lost ~20 us to scalar-queue head-of-line
        # blocking -- the ~9 us serialized drain is cheaper than any SBUF
        # rebalance that was tried against it.
        for sz, bufs in ((2, 2), (8, 4)):
            if sz in x_chunks:
                xpools[sz] = ctx.enter_context(
                    tc.tile_pool(name=f"xpool{sz}", bufs=bufs))
        opool = ctx.enter_context(tc.tile_pool(name="opool", bufs=4))
        psum_o = ctx.enter_context(tc.tile_pool(name="psum_o", bufs=3, space="PSUM"))
        psum_s = ctx.enter_context(tc.tile_pool(name="psum_s", bufs=2, space="PSUM"))
        cpool = ctx.enter_context(tc.tile_pool(name="cpool", bufs=2))

        # ---- Phase 1: e = exp(c), Exp-batched on ScalarE ----
        # The c DMA is split into pieces so the first Exps run as soon as the
        # first piece lands (a single 2 MiB c DMA interleaves with the x
        # stream at descriptor granularity and only completes at ~24 us,
        # which delayed the entire pipeline). Scalar-queue HWDGE: dispatches
        # run concurrently with the x dispatches on the sync queue.
        CPIECE = max(1, n_groups // 4)  # groups per c piece
        c_tiles = []
        # The pieces share one tag (2 rotating buffers): piece k reuses piece
        # k-2's buffer once its Exps have read it, saving 8 KiB of SBUF.
        for p0 in range(0, n_groups, CPIECE):
            ct = cpool.tile([P, CPIECE * G1 * P], BF16, name=f"c{p0}",
                            tag="cpiece")
            nc.scalar.dma_start(
                out=ct[:],
                in_=c[:, p0 * G1 * P:(p0 + CPIECE) * G1 * P],
            )
            c_tiles.append(ct)
            for g in range(p0, p0 + CPIECE):
                nc.scalar.activation(
                    w_tiles[g][:],
                    ct[:, (g - p0) * G1 * P:(g - p0 + 1) * G1 * P],
                    mybir.ActivationFunctionType.Exp)

        # ---- Phase 2: weight-stationary block matmuls over the full batch,
        # with the per-group colsum/reciprocal emitted lazily at group
        # boundaries so no engine queue is dammed up at the start.
        ev = 0
        groups_done = set()
        nb0 = 0
        for XCHUNK in x_chunks:
            gx0 = nb0
            xt = xpools[XCHUNK].tile([P, XCHUNK * batch], BF16)
            nc.sync.dma_start(
                out=xt[:],
                in_=x[:, gx0 * batch:(gx0 + XCHUNK) * batch],
            )
            nb0 += XCHUNK
            for jo in range(XCHUNK // OCHUNK):
                ot = opool.tile([P, OCHUNK * batch], BF16)
                for j2 in range(OCHUNK):
                    j = jo * OCHUNK + j2
                    n = gx0 + j
                    g, r = divmod(n, G1)
                    if g not in groups_done:
                        groups_done.add(g)
                        ps = psum_s.tile([P, G1], F32)
                        for rr in range(G1):
                            nc.tensor.matmul(
                                ps[:, rr:rr + 1],
                                w_slice(g * G1 + rr),
                                ones_sb[:, 0:1],
                                start=True,
                                stop=True,
                            )
                        nc.vector.reciprocal_approx_fast(
                            out=rv_tiles[g][:], in_=ps[:])
                    for h in range(batch // PS_COLS):
                        pso = psum_o.tile([P, PS_COLS], F32)
                        for k in range(PS_COLS // NMM):
                            col = h * PS_COLS + k * NMM
                            nc.tensor.matmul(
                                pso[:, k * NMM:(k + 1) * NMM],
                                w_slice(n),
                                xt[:, j * batch + col:j * batch + col + NMM],
                                start=True,
                                stop=True,
                            )
                        evict(
                            ev,
                            ot[:, j2 * batch + h * PS_COLS:
                               j2 * batch + (h + 1) * PS_COLS],
                            pso[:],
                            rv_tiles[g][:, r:r + 1],
                        )
                        ev += 1
                # Out DMAs never dispatch on the sync queue mid-run: a
                # dma_start dispatch WAITS for its input tile's evictions
                # before generating descriptors, so on the sync queue each
                # out dispatch head-of-line blocked every later x prefetch
                # dispatch behind it (sync carries only x until the final
                # chunk). They alternate between the GpSimd SWDGE queue
                # (idle, ~2 us/dispatch) and the Scalar queue (rides between
                # ACT evictions, which have ~60 us of slack) so neither
                # queue's serialization paces the drain; the final chunk's
                # tile takes the by-then-idle sync queue.
                if nb0 >= blocks:
                    eng = nc.sync
                elif ev % 2 == 0:
                    eng = nc.gpsimd
                else:
                    eng = nc.scalar
                eng.dma_start(
                    out=out[:, (gx0 + jo * OCHUNK) * batch:
                            (gx0 + (jo + 1) * OCHUNK) * batch],
                    in_=ot[:],
                )


def build_program(batch=BATCH, blocks=BLOCKS_PER_CORE):
    nc = bacc.Bacc("TRN2", target_bir_lowering=False, debug=False)
    # x arrives host-repacked as m-major bf16 [m, n*b], see repack_x.
    x = nc.dram_tensor("x", [P, blocks * batch], BF16, kind="ExternalInput").ap()
    # c arrives host-repacked as m-major bf16 [m, n*o], see repack_c.
    c = nc.dram_tensor("c", [P, blocks * P], BF16, kind="ExternalInput").ap()
    # out leaves o-major bf16 [o, n*b], untransposed on host, see unpack_out.
    out = nc.dram_tensor("out", [P, blocks * batch], BF16, kind="ExternalOutput").ap()
    with tile.TileContext(nc) as tc:
        _body(tc, out, x, c, batch, blocks)
    nc.compile()
    return nc


_NC_CACHE = {}


def _get_nc():
    if "nc" not in _NC_CACHE:
        _NC_CACHE["nc"] = build_program()
    return _NC_CACHE["nc"]


def repack_c(c_shard):
    """[n, m, o] -> m-major bf16 [m, n*o]: one efficient DMA, half the bytes.

    bf16 c moves the end-to-end rel err from ~6e-3 to ~1.4e-2 (validated at
    full scale on the exact device arithmetic path) -- still 1.4x under the
    2e-2 budget -- and saves ~6.5 us of DMA."""
    n = c_shard.shape[0]
    return (
        c_shard.transpose(1, 0, 2)
        .astype(BF16_NP)
        .reshape(P, n * P)
    )


def repack_x(x_shard):
    """[b, n*m] fp32 -> m-major bf16 [m, n*b]: contraction dim on partitions."""
    batch, cols = x_shard.shape
    nb = cols // P
    return (
        x_shard.reshape(batch, nb, P)
        .transpose(2, 1, 0)
        .astype(BF16_NP)
        .reshape(P, nb * batch)
    )


def unpack_out(o_packed, batch, blocks):
    """o-major bf16 [o, n*b] -> [b, n*o] fp32."""
    return (
        np.asarray(o_packed)
        .reshape(P, blocks, batch)
        .transpose(2, 1, 0)
        .astype(np.float32)
        .reshape(batch, blocks * P)
    )


def _make_in_maps(x, c):
    xr = x.reshape(BATCH, N_CORES, XCOLS)
    in_maps = []
    for i in range(N_CORES):
        in_maps.append(
            {
                "x": repack_x(np.ascontiguousarray(xr[:, i, :])),
                "c": repack_c(c[i * BLOCKS_PER_CORE:(i + 1) * BLOCKS_PER_CORE]),
            }
        )
    return in_maps


def run_on_hw(x, c, trace=False):
    """Run the SPMD kernel on the 8 cores; returns (out, BassKernelResults)."""
    x = np.asarray(x, dtype=np.float32)
    c = np.asarray(c, dtype=np.float32)
    assert x.shape == (BATCH, LAYER), x.shape
    assert c.shape == (N_BLOCKS_TOTAL, P, P), c.shape
    nc = _get_nc()
    in_maps = _make_in_maps(x, c)
    res = None
    for attempt in range(3):
        try:
            res = run_bass_kernel_spmd(
                nc, in_maps, core_ids=list(range(N_CORES)), trace=trace
            )
            break
        except Exception:
            # Transient runtime failures (e.g. a device flake) are rare but
            # fatal to a single attempt; retry with a fresh dispatch.
            if attempt == 2:
                raise
    assert res is not None
    out = np.empty((BATCH, LAYER), dtype=np.float32)
    orv = out.reshape(BATCH, N_CORES, XCOLS)
    for i in range(N_CORES):
        orv[:, i, :] = unpack_out(res.results[i]["out"], BATCH, BLOCKS_PER_CORE)
    return out, res


def kernel(x, c):
    out, _ = run_on_hw(x, c, trace=False)
    return out
